# revision 1
# baseline (speedup 1.0000x reference)
"""Trainium2 Bass kernel for nn_LocalDiscriminator (patch-GAN style loss).

Reference computation (full shapes):
    x: [32, 1024, 64, 64] f32, w: [1, 1024] f32, b: [1] f32, mode: scalar int
    logits = einsum('bchw,c->bhw', x, w[0]) + b[0]
    z = sigmoid(logits)
    loss = mean(softplus(z) - z * mode)        # scalar f32

Strategy: data-parallel over the batch dim — 4 batches per core on 8 cores.
Each core streams its 64 MiB shard of x through the TensorEngine: the
channel contraction uses lhsT = [w_col, w_col] ([128, 2] stationary, f32r
so the moving data streams at 1 cycle/row instead of fp32's 4), writing
IDENTICAL logits to two PSUM partitions. One ScalarEngine tanh per
[2, 2048] group — with per-partition scale/bias APs — then evaluates both
reductions at once, and its accum_out port emits the per-group sums for
free (one ACT LUT table for the whole kernel; the per-group sums go
straight to DRAM and the final reduction happens on the host):
    partition 0:  sum tanh(FA*t + FA*b+FB)   -> softplus fit
    partition 1:  sum tanh(0.5*t + 0.5*b)    -> exact sigmoid identity
where t is the raw logit. Host combination:
    sum(z)            = N/2 + S_z/2                             (exact)
    sum(softplus(z)) ~= N*FC0 + FC1*S_f                         (fitted)
    loss = (sum(softplus(z)) - mode*sum(z)) / N
The fit softplus(sigmoid(t)) ~= FC0 + FC1*tanh(FA*t+FB) has max |err|
9.8e-4 per element on t in [-4.5, 4.5] and its mean error over the
N(0, ~0.64) logit distribution cancels to ~1e-7 — the loss is a mean of
131072 such elements, so even the worst-case systematic error (1.4e-3,
saturated logits) is 14x inside the 2e-2 gate.
The kernel is HBM-bandwidth-bound: ~64 MiB/core at ~358 GB/s (~187 us);
the TimelineSim cost model predicts 201 us/core end-to-end (the gap is
the fixed start/exit barriers plus the last chunk's gated tail work,
which is minimized by staging the final chunk-7 DMA in halves and
emitting the last batch's matmuls contraction-chunk-outer so the
in-order PE stream never queues stalled work ahead of ready work).
"""

import os
import sys

import numpy as np

_REPO_CANDIDATES = ("/opt/trn_rl_repo", "/root/.axon_site/_ro/trn_rl_repo")
for _p in _REPO_CANDIDATES:
    if os.path.isdir(_p) and _p not in sys.path:
        sys.path.insert(0, _p)

import concourse.bacc as bacc
import concourse.bass as bass
import concourse.mybir as mybir
import concourse.tile as tile
from concourse.bass_utils import run_bass_kernel_spmd

N_CORES = 8
B_FULL, C, H, W = 32, 1024, 64, 64
B_LOCAL = B_FULL // N_CORES          # 4 batches per core
HW = H * W                           # 4096 spatial positions per batch
C_CHUNKS = C // 128                  # 8 chunks of 128 channels
BANKS = 4                            # psum banks per group: [2, 4, 512]
GRP = BANKS * 512                    # 2048 positions per pointwise group
GRPS = HW // GRP                     # 2 groups per batch
N_LOCAL = B_LOCAL * HW               # positions per core

# softplus(sigmoid(t)) ~= FC0 + FC1 * tanh(FA*t + FB)
FC0 = 1.0028824947566075
FC1 = 0.30899789558232016
FA = 0.5078652298016119
FB = -0.09351045988102749

F32 = mybir.dt.float32
F32R = mybir.dt.float32r

_nc_cache = None
_exec_cache = None


def _build_nc():
    nc = bacc.Bacc("TRN2", target_bir_lowering=False, debug=False,
                   num_devices=N_CORES)

    x = nc.dram_tensor("x", [B_LOCAL, C, H, W], F32, kind="ExternalInput").ap()
    w = nc.dram_tensor("w", [1, C], F32, kind="ExternalInput").ap()
    # aff[p] = (scale, bias) for the tanh on psum partition p; computed on
    # the host from the Linear bias b:
    #   row 0 = (FA, FA*b+FB)   (softplus fit), row 1 = (0.5, 0.5*b) (sigmoid)
    aff = nc.dram_tensor("aff", [2, 2], F32, kind="ExternalInput").ap()
    # raw per-group tanh sums; the final reduction happens on the host
    # (saves a DVE reduce + sem round-trip off the kernel tail).
    N_GROUPS = B_LOCAL * 2
    out = nc.dram_tensor("out", [2, N_GROUPS], F32,
                         kind="ExternalOutput").ap()

    xr = x.rearrange("b c h w -> b c (h w)")  # [B_LOCAL, 1024, 4096]

    with tile.TileContext(nc) as tc:
        with (
            tc.tile_pool(name="xpool", bufs=6) as xpool,
            tc.tile_pool(name="const", bufs=1) as cpool,
            tc.tile_pool(name="sums", bufs=1) as spool,
            tc.tile_pool(name="dump", bufs=1) as dpool,
            tc.tile_pool(name="psum", bufs=2, space="PSUM") as pspool,
        ):
            # Two copies of w side by side: lhsT [128, 2] per chunk makes the
            # matmul write identical logits to TWO psum partitions, so one
            # ACT tanh with per-partition scale/bias evaluates both the
            # softplus fit (partition 0) and the exact sigmoid identity
            # (partition 1) in a single instruction.
            # w2[p, k, j] = w[0, j*128 + p] for k in {0,1}.
            w2 = cpool.tile([128, 2, C_CHUNKS], F32R, tag="w")
            nc.gpsimd.dma_start(
                out=w2[:, 0, :],
                in_=w[0].bitcast(F32R).rearrange("(j p) -> p j", p=128))
            # replicate the second stationary copy on the idle VectorE
            # instead of paying a second scattered DMA on the shared engines
            nc.vector.tensor_copy(w2[:, 1, :], w2[:, 0, :])
            aff_t = cpool.tile([2, 2], F32, tag="aff")
            nc.gpsimd.dma_start(out=aff_t[:], in_=aff[:])

            # sums[0, i] = sum tanh(FA*t+FB') of group i  (softplus fit)
            # sums[1, i] = sum tanh(t/2+b/2) of group i   (sigmoid)
            sums = spool.tile([2, N_GROUPS], F32, tag="sums")

            for bi in range(B_LOCAL):
                # Load channel-chunk PAIRS: one contiguous 4 MiB block per
                # dma_start ([128, 2*HW] tile; chunk c at columns
                # (c%2)*HW : (c%2+1)*HW).  4 MiB transfers clear the ~91%
                # DMA-efficiency knee, so the stream runs at the HBM ceiling.
                # The last batch's final pair is split in two so the tail
                # dependency (chunk 7) lands sooner.
                last = bi == B_LOCAL - 1
                chunk_slices = {}
                for p in range(C_CHUNKS // 2):
                    xt = xpool.tile([128, 2 * HW], F32R, tag="x")
                    if last and p == C_CHUNKS // 2 - 1:
                        # stage the final pair so the end-of-kernel dependency
                        # chain shrinks: BOTH c6 and c7 in progressively
                        # smaller hw pieces matching the group splits — each
                        # piece gates only its group's matmuls, which overlap
                        # the next piece's DMA.
                        nc.sync.dma_start(
                            out=xt[:, :HW],
                            in_=xr[bi, bass.ts(2 * p, 128), :].bitcast(F32R))
                        for hw0, ncols in ((0, 2048), (2048, 1536),
                                           (3584, 512)):
                            nc.sync.dma_start(
                                out=xt[:, HW + hw0:HW + hw0 + ncols],
                                in_=xr[bi, bass.ts(2 * p + 1, 128),
                                       hw0:hw0 + ncols].bitcast(F32R))
                    else:
                        nc.sync.dma_start(
                            out=xt[:],
                            in_=xr[bi, 256 * p:256 * (p + 1), :].bitcast(F32R))
                    chunk_slices[2 * p] = (xt, 0)
                    chunk_slices[2 * p + 1] = (xt, HW)
                def emit_act(ps, jj0, nbank, ncols, idx):
                    # Only the accum_out sums are consumed; the elementwise
                    # tanh output goes to a scratch tile.
                    dump = dpool.tile([2, GRP], F32, tag="dump")
                    nc.scalar.activation(
                        dump[:2, :ncols],
                        ps[0:2, jj0:jj0 + nbank, :].rearrange("p a b -> p (a b)"),
                        mybir.ActivationFunctionType.Tanh,
                        bias=aff_t[:, 1:2], scale=aff_t[:, 0:1],
                        accum_out=sums[0:2, idx:idx + 1],
                    )

                def emit_mm(ps, jj, xt, off, col0, c):
                    nc.tensor.matmul(
                        ps[0:2, jj, :],
                        lhsT=w2[:, :, c],
                        rhs=xt[:, off + col0:off + col0 + 512],
                        start=(c == 0),
                        stop=(c == C_CHUNKS - 1),
                    )

                # Every batch uses the same four single-buffered psum tags
                # sized [4, 2, 1, 1] banks (= all 8 PSUM banks), so the last
                # batch can hold all four of its groups open at once.
                GROUPS = ((0, 2048, "t4a"), (2048, 2048, "t4b"))
                ps_list = []
                for gi, (_, ncols, tg) in enumerate(GROUPS):
                    ps_g = pspool.tile([2, ncols // 512, 512], F32,
                                       name=f"ps_{bi}_{gi}", tag=tg, bufs=1)
                    ps_list.append(ps_g)
                if not last:
                    for gi, (gcol0, ncols, _) in enumerate(GROUPS):
                        for jj in range(ncols // 512):
                            for c in range(C_CHUNKS):
                                xt, off = chunk_slices[c]
                                emit_mm(ps_list[gi], jj, xt, off,
                                        gcol0 + jj * 512, c)
                        emit_act(ps_list[gi], 0, ncols // 512, ncols,
                                 bi * len(GROUPS) + gi)
                else:
                    # Last batch: the PE stream is in-order, so emit
                    # c0..c5 matmuls first (their data arrived long ago),
                    # keeping all four groups' psum banks open, then drain
                    # c6 and c7 piece by piece as their staged DMAs land.
                    for c in range(C_CHUNKS - 1):
                        xt, off = chunk_slices[c]
                        for gi, (gcol0, ncols, _) in enumerate(GROUPS):
                            for jj in range(ncols // 512):
                                emit_mm(ps_list[gi], jj, xt, off,
                                        gcol0 + jj * 512, c)
                    xt, off = chunk_slices[C_CHUNKS - 1]
                    for gi, (gcol0, ncols, _) in enumerate(GROUPS):
                        for jj in range(ncols // 512):
                            emit_mm(ps_list[gi], jj, xt, off,
                                    gcol0 + jj * 512, C_CHUNKS - 1)
                        emit_act(ps_list[gi], 0, ncols // 512, ncols,
                                 bi * len(GROUPS) + gi)

            nc.sync.dma_start(out=out[:], in_=sums[:])

    nc.compile()
    return nc


def _get_nc():
    global _nc_cache
    if _nc_cache is None:
        _nc_cache = _build_nc()
    return _nc_cache


def _get_exec():
    """Compile the 8-core SPMD executable once and cache the jitted callable
    (run_bass_kernel_spmd rebuilds + recompiles the NEFF on every call)."""
    global _exec_cache
    if _exec_cache is not None:
        return _exec_cache

    import jax
    import concourse.mybir as _mybir
    from concourse import bass2jax
    from jax.experimental.shard_map import shard_map
    from jax.sharding import Mesh, PartitionSpec

    nc = _get_nc()
    bass2jax.install_neuronx_cc_hook()

    partition_name = (nc.partition_id_tensor.name
                      if nc.partition_id_tensor else None)
    in_names, out_names, out_avals = [], [], []
    for alloc in nc.m.functions[0].allocations:
        if not isinstance(alloc, _mybir.MemoryLocationSet):
            continue
        name = alloc.memorylocations[0].name
        if alloc.kind == "ExternalInput":
            if name != partition_name:
                in_names.append(name)
        elif alloc.kind == "ExternalOutput":
            shape = tuple(alloc.tensor_shape)
            dtype = _mybir.dt.np(alloc.dtype)
            out_names.append(name)
            out_avals.append(jax.core.ShapedArray(shape, dtype))
    n_params = len(in_names)
    all_in_names = list(in_names) + list(out_names)
    if partition_name is not None:
        all_in_names.append(partition_name)

    def _body(*args):
        operands = list(args)
        if partition_name is not None:
            operands.append(bass2jax.partition_id_tensor())
        outs = bass2jax._bass_exec_p.bind(
            *operands,
            out_avals=tuple(out_avals),
            in_names=tuple(all_in_names),
            out_names=tuple(out_names),
            lowering_input_output_aliases=(),
            sim_require_finite=True,
            sim_require_nnan=True,
            nc=nc,
        )
        return tuple(outs)

    devices = jax.devices()[:N_CORES]
    mesh = Mesh(np.asarray(devices), ("core",))
    n_outs = len(out_names)
    sharded = jax.jit(
        shard_map(
            _body, mesh=mesh,
            in_specs=(PartitionSpec("core"),) * (n_params + n_outs),
            out_specs=(PartitionSpec("core"),) * n_outs,
            check_rep=False,
        ),
        donate_argnums=tuple(range(n_params, n_params + n_outs)),
        keep_unused=True,
    )
    _exec_cache = (sharded, in_names, out_names, out_avals)
    return _exec_cache


def _run_spmd(in_maps):
    """Run the cached executable; returns list of per-core output dicts."""
    sharded, in_names, out_names, out_avals = _get_exec()
    concat_in = [
        np.concatenate([np.asarray(m[name]) for m in in_maps], axis=0)
        for name in in_names
    ]
    concat_zeros = [
        np.zeros((N_CORES * av.shape[0], *av.shape[1:]), av.dtype)
        for av in out_avals
    ]
    out_arrs = sharded(*concat_in, *concat_zeros)
    return [
        {name: np.asarray(out_arrs[i]).reshape(N_CORES, *out_avals[i].shape)[c]
         for i, name in enumerate(out_names)}
        for c in range(N_CORES)
    ]


def kernel(x: np.ndarray, w: np.ndarray, b: np.ndarray, mode) -> np.ndarray:
    x = np.ascontiguousarray(np.asarray(x, dtype=np.float32))
    w = np.ascontiguousarray(np.asarray(w, dtype=np.float32))
    b = np.ascontiguousarray(np.asarray(b, dtype=np.float32))
    assert x.shape == (B_FULL, C, H, W), x.shape

    b0 = float(b.reshape(-1)[0])
    aff = np.array([[FA, FA * b0 + FB], [0.5, 0.5 * b0]], dtype=np.float32)
    in_maps = [
        {"x": x[i * B_LOCAL:(i + 1) * B_LOCAL], "w": w, "aff": aff}
        for i in range(N_CORES)
    ]
    try:
        results = _run_spmd(in_maps)
    except Exception:
        nc = _get_nc()
        results = run_bass_kernel_spmd(nc, in_maps, list(range(N_CORES))).results
    partial = np.stack([r["out"] for r in results])  # [8, 2, 16] group sums

    n_total = float(B_FULL * HW)
    sum_f = float(partial[:, 0, :].sum())
    sum_z = float(partial[:, 1, :].sum())
    s_sp = n_total * FC0 + FC1 * sum_f
    s_z = n_total / 2.0 + sum_z / 2.0
    y = float(np.asarray(mode))
    loss = (s_sp - y * s_z) / n_total
    return np.float32(loss)



# revision 4
# speedup vs baseline: 1.0366x; 1.0366x over previous
"""Trainium2 Bass kernel for nn_LocalDiscriminator (patch-GAN style loss).

Reference computation (full shapes):
    x: [32, 1024, 64, 64] f32, w: [1, 1024] f32, b: [1] f32, mode: scalar int
    logits = einsum('bchw,c->bhw', x, w[0]) + b[0]
    z = sigmoid(logits)
    loss = mean(softplus(z) - z * mode)        # scalar f32

Strategy: data-parallel over the batch dim — 4 batches per core on 8 cores.
Each core streams its 64 MiB shard of x through the TensorEngine: the
channel contraction uses lhsT = [w_col, w_col] ([128, 2] stationary, f32r
so the moving data streams at 1 cycle/row instead of fp32's 4), writing
IDENTICAL logits to two PSUM partitions. One ScalarEngine tanh per group —
with per-partition scale/bias APs — evaluates both reductions at once, and
its accum_out port emits the per-group sums for free (one ACT LUT table for
the whole kernel; the per-group sums go straight to DRAM and the final
reduction happens on the host):
    partition 0:  sum tanh(FA*t + FA*b+FB)   -> softplus fit
    partition 1:  sum tanh(0.5*t + 0.5*b)    -> exact sigmoid identity
where t is the raw logit. Host combination:
    sum(z)            = N/2 + S_z/2                             (exact)
    sum(softplus(z)) ~= N*FC0 + FC1*S_f                         (fitted)
    loss = (sum(softplus(z)) - mode*sum(z)) / N
The fit softplus(sigmoid(t)) ~= FC0 + FC1*tanh(FA*t+FB) has max |err|
9.8e-4 per element on t in [-4.5, 4.5] and its mean error over the
N(0, ~0.64) logit distribution cancels to ~1e-7 — the loss is a mean of
131072 such elements, so even the worst-case systematic error (1.4e-3,
saturated logits) is 14x inside the 2e-2 gate.

The kernel is HBM-bandwidth-bound: ~64 MiB/core at 360 GB/s (~186.4 us of
pure transfer). Two things keep the end-to-end time near that floor:
  * Channel fold c = 8q + i (chunk i puts channel 8q+i on partition q):
    the w scatter then has 32 B descriptors (56 ns of DMA time) instead of
    4 B ones (448 ns), and each x pair-load still moves two ADJACENT
    channel rows per partition = 32 KiB contiguous descriptors.
  * The last batch streams COLUMN-major in three waves (cols 0:2048,
    2048:3584, 3584:4096 = 4+3+1 psum banks), so the work gated on the
    final DMA piece (128 KiB, 256 cols of chunk 7) is one [2, 256] matmul,
    one 512-col activation, and the 128 B result DMA — the tail past the
    last x byte is ~5 us instead of ~12 us (matmuls for a whole trailing
    chunk plus two full 2048-col activations).
"""

import os
import sys

import numpy as np

_REPO_CANDIDATES = ("/opt/trn_rl_repo", "/root/.axon_site/_ro/trn_rl_repo")
for _p in _REPO_CANDIDATES:
    if os.path.isdir(_p) and _p not in sys.path:
        sys.path.insert(0, _p)

import concourse.bacc as bacc
import concourse.bass as bass
import concourse.mybir as mybir
import concourse.tile as tile
from concourse.bass_utils import run_bass_kernel_spmd

N_CORES = 8
B_FULL, C, H, W = 32, 1024, 64, 64
B_LOCAL = B_FULL // N_CORES          # 4 batches per core
HW = H * W                           # 4096 spatial positions per batch
C_CHUNKS = C // 128                  # 8 chunks of 128 channels
N_GROUPS = (B_LOCAL - 1) * 2 + 3     # 2 act-groups/batch + 3 on the last

# softplus(sigmoid(t)) ~= FC0 + FC1 * tanh(FA*t + FB)
FC0 = 1.0028824947566075
FC1 = 0.30899789558232016
FA = 0.5078652298016119
FB = -0.09351045988102749

F32 = mybir.dt.float32
F32R = mybir.dt.float32r

_nc_cache = None
_exec_cache = None


def _build_nc():
    nc = bacc.Bacc("TRN2", target_bir_lowering=False, debug=False,
                   num_devices=N_CORES)

    x = nc.dram_tensor("x", [B_LOCAL, C, H, W], F32, kind="ExternalInput").ap()
    w = nc.dram_tensor("w", [1, C], F32, kind="ExternalInput").ap()
    # aff[p] = (scale, bias) for the tanh on psum partition p; computed on
    # the host from the Linear bias b:
    #   row 0 = (FA, FA*b+FB)   (softplus fit), row 1 = (0.5, 0.5*b) (sigmoid)
    aff = nc.dram_tensor("aff", [2, 2], F32, kind="ExternalInput").ap()
    # raw per-group tanh sums; the final reduction happens on the host
    # (saves a DVE reduce + sem round-trip off the kernel tail).
    out = nc.dram_tensor("out", [2, N_GROUPS], F32,
                         kind="ExternalOutput").ap()

    # Channel fold: chunk i holds channels {8q + i}, so partition q of a
    # chunk-pair tile reads two ADJACENT 16 KiB channel rows (32 KiB
    # contiguous descriptors) and w folds to [128, 8] with 32 B descriptors.
    xq = x.rearrange("b (q t) h w -> b q t (h w)", t=C_CHUNKS)

    with tile.TileContext(nc) as tc:
        with (
            tc.tile_pool(name="xpool", bufs=6) as xpool,
            tc.tile_pool(name="const", bufs=1) as cpool,
            tc.tile_pool(name="sums", bufs=1) as spool,
            tc.tile_pool(name="dump", bufs=1) as dpool,
            tc.tile_pool(name="psum", bufs=2, space="PSUM") as pspool,
        ):
            # Two copies of w side by side: lhsT [128, 2] per chunk makes the
            # matmul write identical logits to TWO psum partitions, so one
            # ACT tanh with per-partition scale/bias evaluates both the
            # softplus fit (partition 0) and the exact sigmoid identity
            # (partition 1) in a single instruction.
            # w2[q, k, i] = w[0, 8*q + i] for k in {0,1}.
            w2 = cpool.tile([128, 2, C_CHUNKS], F32R, tag="w")
            nc.gpsimd.dma_start(
                out=w2[:, 0, :],
                in_=w[0].bitcast(F32R).rearrange("(p i) -> p i", p=128))
            # replicate the second stationary copy on the idle VectorE
            # instead of paying a second scattered DMA on the shared engines
            nc.vector.tensor_copy(w2[:, 1, :], w2[:, 0, :])
            aff_t = cpool.tile([2, 2], F32, tag="aff")
            nc.gpsimd.dma_start(out=aff_t[:], in_=aff[:])

            # sums[0, i] = sum tanh(FA*t+FB') of group i  (softplus fit)
            # sums[1, i] = sum tanh(t/2+b/2) of group i   (sigmoid)
            sums = spool.tile([2, N_GROUPS], F32, tag="sums")

            def emit_act(ps, nbank, ncols, idx):
                # Only the accum_out sums are consumed; the elementwise
                # tanh output goes to a scratch tile.
                dump = dpool.tile([2, 2048], F32, tag="dump")
                nc.scalar.activation(
                    dump[:2, :ncols],
                    ps[0:2, 0:nbank, :].rearrange("p a b -> p (a b)"),
                    mybir.ActivationFunctionType.Tanh,
                    bias=aff_t[:, 1:2], scale=aff_t[:, 0:1],
                    accum_out=sums[0:2, idx:idx + 1],
                )

            def emit_mm(ps, jj, rhs, c, ncols=512, colofs=0):
                nc.tensor.matmul(
                    ps[0:2, jj, colofs:colofs + ncols],
                    lhsT=w2[:, :, c],
                    rhs=rhs,
                    start=(c == 0),
                    stop=(c == C_CHUNKS - 1),
                )

            # Batches 0..B_LOCAL-2: stream chunk-pair-major (4 MiB loads),
            # two 2048-col groups per batch on all 8 psum banks.
            for bi in range(B_LOCAL - 1):
                tiles = []
                for p in range(C_CHUNKS // 2):
                    xt = xpool.tile([128, 2, HW], F32R, tag="x",
                                    name=f"xt_{bi}_{p}")
                    nc.sync.dma_start(
                        out=xt[:],
                        in_=xq[bi, :, 2 * p:2 * p + 2, :].bitcast(F32R))
                    tiles.append(xt)
                for gi, tg in enumerate(("t4a", "t4b")):
                    ps_g = pspool.tile([2, 4, 512], F32,
                                       name=f"ps_{bi}_{gi}", tag=tg, bufs=1)
                    for c in range(C_CHUNKS):
                        xt = tiles[c // 2]
                        for jj in range(4):
                            col = gi * 2048 + jj * 512
                            emit_mm(ps_g, jj, xt[:, c % 2, col:col + 512], c)
                    emit_act(ps_g, 4, 2048, bi * 2 + gi)

            # Last batch: stream COLUMN-major in three waves so each act
            # group completes (and its ACT runs) while later columns are
            # still in flight; the final wave's last piece is only 128 KiB.
            bi = B_LOCAL - 1
            tiles = [xpool.tile([128, 2, HW], F32R, tag="x",
                                name=f"xt_last_{p}")
                     for p in range(C_CHUNKS // 2)]
            # (col0, ncols, psum tag, group index). g2 reuses t4a: its
            # banks are free once g0's ACT has read them (~20 us earlier).
            WAVES = ((0, 2048, "t4a", 6), (2048, 1536, "t4b", 7),
                     (3584, 512, "t4a", 8))
            for c0, ncols, tg, idx in WAVES:
                nbank = ncols // 512
                ps_g = pspool.tile([2, 4, 512], F32,
                                   name=f"ps_last_{idx}", tag=tg, bufs=1)
                last_wave = c0 == 3584
                for p in range(C_CHUNKS // 2):
                    xt = tiles[p]
                    if not (last_wave and p == C_CHUNKS // 2 - 1):
                        nc.sync.dma_start(
                            out=xt[:, :, c0:c0 + ncols],
                            in_=xq[bi, :, 2 * p:2 * p + 2,
                                   c0:c0 + ncols].bitcast(F32R))
                        for h in range(2):
                            for jj in range(nbank):
                                col = c0 + jj * 512
                                emit_mm(ps_g, jj,
                                        xt[:, h, col:col + 512], 2 * p + h)
                    else:
                        # Final pair (chunks 6, 7) of the 512-col wave:
                        # c6, then c7 in two 256-col halves so the work
                        # gated on the very last DMA is a single [2, 256]
                        # matmul.
                        nc.sync.dma_start(
                            out=xt[:, 0, c0:c0 + 512],
                            in_=xq[bi, :, 6, c0:c0 + 512].bitcast(F32R))
                        emit_mm(ps_g, 0, xt[:, 0, c0:c0 + 512], 6)
                        for half in range(2):
                            lo = c0 + 256 * half
                            nc.sync.dma_start(
                                out=xt[:, 1, lo:lo + 256],
                                in_=xq[bi, :, 7, lo:lo + 256].bitcast(F32R))
                            emit_mm(ps_g, 0, xt[:, 1, lo:lo + 256], 7,
                                    ncols=256, colofs=256 * half)
                emit_act(ps_g, nbank, ncols, idx)

            nc.sync.dma_start(out=out[:], in_=sums[:])

    nc.compile()
    return nc


def _get_nc():
    global _nc_cache
    if _nc_cache is None:
        _nc_cache = _build_nc()
    return _nc_cache


def _get_exec():
    """Compile the 8-core SPMD executable once and cache the jitted callable
    (run_bass_kernel_spmd rebuilds + recompiles the NEFF on every call)."""
    global _exec_cache
    if _exec_cache is not None:
        return _exec_cache

    import jax
    import concourse.mybir as _mybir
    from concourse import bass2jax
    from jax.experimental.shard_map import shard_map
    from jax.sharding import Mesh, PartitionSpec

    nc = _get_nc()
    bass2jax.install_neuronx_cc_hook()

    partition_name = (nc.partition_id_tensor.name
                      if nc.partition_id_tensor else None)
    in_names, out_names, out_avals = [], [], []
    for alloc in nc.m.functions[0].allocations:
        if not isinstance(alloc, _mybir.MemoryLocationSet):
            continue
        name = alloc.memorylocations[0].name
        if alloc.kind == "ExternalInput":
            if name != partition_name:
                in_names.append(name)
        elif alloc.kind == "ExternalOutput":
            shape = tuple(alloc.tensor_shape)
            dtype = _mybir.dt.np(alloc.dtype)
            out_names.append(name)
            out_avals.append(jax.core.ShapedArray(shape, dtype))
    n_params = len(in_names)
    all_in_names = list(in_names) + list(out_names)
    if partition_name is not None:
        all_in_names.append(partition_name)

    def _body(*args):
        operands = list(args)
        if partition_name is not None:
            operands.append(bass2jax.partition_id_tensor())
        outs = bass2jax._bass_exec_p.bind(
            *operands,
            out_avals=tuple(out_avals),
            in_names=tuple(all_in_names),
            out_names=tuple(out_names),
            lowering_input_output_aliases=(),
            sim_require_finite=True,
            sim_require_nnan=True,
            nc=nc,
        )
        return tuple(outs)

    devices = jax.devices()[:N_CORES]
    mesh = Mesh(np.asarray(devices), ("core",))
    n_outs = len(out_names)
    sharded = jax.jit(
        shard_map(
            _body, mesh=mesh,
            in_specs=(PartitionSpec("core"),) * (n_params + n_outs),
            out_specs=(PartitionSpec("core"),) * n_outs,
            check_rep=False,
        ),
        donate_argnums=tuple(range(n_params, n_params + n_outs)),
        keep_unused=True,
    )
    _exec_cache = (sharded, in_names, out_names, out_avals)
    return _exec_cache


def _run_spmd(in_maps):
    """Run the cached executable; returns list of per-core output dicts."""
    sharded, in_names, out_names, out_avals = _get_exec()
    concat_in = [
        np.concatenate([np.asarray(m[name]) for m in in_maps], axis=0)
        for name in in_names
    ]
    concat_zeros = [
        np.zeros((N_CORES * av.shape[0], *av.shape[1:]), av.dtype)
        for av in out_avals
    ]
    out_arrs = sharded(*concat_in, *concat_zeros)
    return [
        {name: np.asarray(out_arrs[i]).reshape(N_CORES, *out_avals[i].shape)[c]
         for i, name in enumerate(out_names)}
        for c in range(N_CORES)
    ]


def kernel(x: np.ndarray, w: np.ndarray, b: np.ndarray, mode) -> np.ndarray:
    x = np.ascontiguousarray(np.asarray(x, dtype=np.float32))
    w = np.ascontiguousarray(np.asarray(w, dtype=np.float32))
    b = np.ascontiguousarray(np.asarray(b, dtype=np.float32))
    assert x.shape == (B_FULL, C, H, W), x.shape

    b0 = float(b.reshape(-1)[0])
    aff = np.array([[FA, FA * b0 + FB], [0.5, 0.5 * b0]], dtype=np.float32)
    in_maps = [
        {"x": x[i * B_LOCAL:(i + 1) * B_LOCAL], "w": w, "aff": aff}
        for i in range(N_CORES)
    ]
    try:
        results = _run_spmd(in_maps)
    except Exception:
        nc = _get_nc()
        results = run_bass_kernel_spmd(nc, in_maps, list(range(N_CORES))).results
    partial = np.stack([r["out"] for r in results])  # [8, 2, N_GROUPS]

    n_total = float(B_FULL * HW)
    sum_f = float(partial[:, 0, :].sum())
    sum_z = float(partial[:, 1, :].sum())
    s_sp = n_total * FC0 + FC1 * sum_f
    s_z = n_total / 2.0 + sum_z / 2.0
    y = float(np.asarray(mode))
    loss = (s_sp - y * s_z) / n_total
    return np.float32(loss)
